# revision 24
# baseline (speedup 1.0000x reference)
"""BiLSTM (2-layer, bidirectional) encoder + attention pooling on 8 Trainium2 cores.

Flat self-timed SPMD pipeline (no tc.If blocks at all):
  core 0: layer-0 forward LSTM  (reads x, full batch 128)
  core 1: layer-1 forward LSTM + f-side attention (consumes core0 via pair-AG)
  core 2: layer-0 backward LSTM (host-reversed x)
  core 3: layer-1 backward LSTM + b-side attention
  cores 4-7: junk mirrors (participate in dummy AG pairs only)

Every core runs the *identical* instruction stream; role asymmetry is purely
data-driven: per-core weights in in_maps, plus predicated DMAs
(dma_start(cond=...)) that select input source (x vs AllGathered h) and
role-shifted store indices. Layer handoff ships transposed hidden states
(bf16) through per-unit pairwise AllGathers ([[0,1],[2,3],[4,5],[6,7]]);
the attention projections (64-dim) are exchanged between cores 1 and 3
([[1,3],...]), so each L1 core accumulates its own half of the attention
output from its locally stored h states (middle-out streaming softmax with
a fixed exp-shift M = sum(max(2*Wa2, 0)); all transcendentals are in the
sigmoid table set: Sigmoid + Tanh).
"""
import numpy as np

B, S_FULL, D, H, A = 128, 512, 256, 256, 64
U = 16            # steps per pipeline unit (AG granularity)
LAG = 2           # units of layer-0 -> layer-1 pipeline lag

_BUILD_CACHE = {}
_last_in_maps = None


def _build(S):
    import concourse.bass as bass
    from concourse import bacc
    import concourse.mybir as mybir
    from concourse.tile import TileContext
    from concourse.masks import make_identity

    F32 = mybir.dt.float32
    F32R = mybir.dt.float32r
    BF16 = mybir.dt.bfloat16
    AF = mybir.ActivationFunctionType
    OP = mybir.AluOpType
    AX = mybir.AxisListType

    NU = S // U          # real units
    NP = NU + LAG        # step positions (incl. warmup/pad junk units)
    NW = NU // 2         # attention windows
    MID = S // 2
    WPOS0 = NU // 2 + LAG + 2   # position at which window 0 is processed

    G1 = [[0, 1], [2, 3], [4, 5], [6, 7]]   # L0 -> L1 h handoff pairs
    G2 = [[0, 2], [1, 3], [4, 6], [5, 7]]   # attention u/v exchange pairs

    nc = bacc.Bacc("TRN2", target_bir_lowering=False, debug=False, num_devices=8)

    xs_d = nc.dram_tensor("xs", [NP * U, 2, 128, B], BF16, kind="ExternalInput")
    wih_d = nc.dram_tensor("wih", [128, 2, 2, 512], BF16, kind="ExternalInput")
    whh_d = nc.dram_tensor("whh", [128, 2, 2, 512], F32, kind="ExternalInput")
    bias_d = nc.dram_tensor("bias", [1, 1024], F32, kind="ExternalInput")
    wa1_d = nc.dram_tensor("wa1s", [128, 2, A], F32, kind="ExternalInput")
    wa2_d = nc.dram_tensor("wa2r", [1, A], F32, kind="ExternalInput")
    mneg_d = nc.dram_tensor("mneg", [1, 1], F32, kind="ExternalInput")
    yout_d = nc.dram_tensor("yout", [B, H], F32, kind="ExternalOutput")
    hlast_d = nc.dram_tensor("hlast", [128, 256], F32, kind="ExternalOutput")
    den_d = nc.dram_tensor("dend", [128, 1], F32, kind="ExternalOutput")
    acc_d = nc.dram_tensor("accd", [128, 256], F32, kind="ExternalOutput")
    agdbg_d = nc.dram_tensor("agdbg", [128, 256], F32, kind="ExternalOutput")
    uxdbg_d = nc.dram_tensor("uxdbg", [128, 256], F32, kind="ExternalOutput")
    a2dbg_d = nc.dram_tensor("a2dbg", [128, 2, 16, A], F32, kind="ExternalOutput")
    zdbg_d = nc.dram_tensor("zdbg", [128, 16], F32, kind="ExternalOutput")
    edbg_d = nc.dram_tensor("edbg", [128, 512], F32, kind="ExternalOutput")
    asdbg_d = nc.dram_tensor("asdbg", [128, 16, A], F32, kind="ExternalOutput")

    agin1 = [nc.dram_tensor(f"agin1_{q}", [U, 128, 256], BF16) for q in range(NU)]
    agbuf1 = [nc.dram_tensor(f"agbuf1_{q}", [2 * U, 128, 256], BF16) for q in range(NU)]
    agin2 = [nc.dram_tensor(f"agin2_{q}", [128, U, A], BF16) for q in range(NU)]
    agbuf2 = [nc.dram_tensor(f"agbuf2_{q}", [2, 128, U, A], BF16) for q in range(NU)]
    hstore = nc.dram_tensor("hstore", [S, 128, 256], F32)
    zdram = nc.dram_tensor("zdram", [128, 512], F32)

    with TileContext(nc) as tc:
        with tc.tile_pool(name="wpool", bufs=1) as wp, \
             tc.tile_pool(name="upool", bufs=2) as up, \
             tc.tile_pool(name="gpool", bufs=3) as gp, \
             tc.tile_pool(name="apool", bufs=2) as ap_, \
             tc.tile_pool(name="hpool", bufs=4) as hp, \
             tc.tile_pool(name="psum", bufs=1, space="PSUM") as pp:

            pid = nc.partition_id()
            sel = pid % 2            # 0: L0-style (input = xs); 1: L1-style
            opos = (pid // 2) % 2    # slot within the G2 pair

            # ---------------- prologue ----------------
            wih = wp.tile([128, 2, 2, 512], BF16, tag="wih")
            nc.sync.dma_start(out=wih[:], in_=wih_d.ap())
            whh = wp.tile([128, 2, 2, 512], F32R, tag="whh")
            nc.gpsimd.dma_start(out=whh[:], in_=whh_d.ap())
            bias_r = wp.tile([1, 1024], BF16, tag="bias_r")
            nc.gpsimd.dma_start(out=bias_r[:], in_=bias_d.ap())
            wa1 = wp.tile([128, 2, A], F32R, tag="wa1")
            nc.gpsimd.dma_start(out=wa1[:], in_=wa1_d.ap())
            wa2bc = wp.tile([128, A], F32, tag="wa2bc")
            nc.sync.dma_start(out=wa2bc[:], in_=wa2_d.ap().partition_broadcast(128))
            mneg = wp.tile([128, 1], F32, tag="mneg")
            nc.sync.dma_start(out=mneg[:], in_=mneg_d.ap().partition_broadcast(128))
            ones_f = wp.tile([1, 128], F32, tag="ones_f")
            nc.gpsimd.memset(ones_f[:], 1.0)
            ones_r = wp.tile([1, 128], BF16, tag="ones_r")
            nc.vector.tensor_copy(out=ones_r[:], in_=ones_f[:])
            ident = wp.tile([128, 128], F32, tag="ident")
            make_identity(nc, ident[:])

            zsb = wp.tile([128, 512], F32, tag="zsb")
            nc.gpsimd.memset(zsb[:], 0.0)
            nc.sync.dma_start(out=zdram.ap(), in_=zsb[:])
            hT_s = wp.tile([128, 2, 128], F32R, tag="hT_s")
            nc.vector.tensor_copy(out=hT_s[:].rearrange("p a b -> p (a b)"),
                                  in_=zsb[:, 0:256])
            c_s = wp.tile([128, 256], F32, tag="c_s")
            nc.vector.tensor_copy(out=c_s[:], in_=zsb[:, 0:256])
            acc = wp.tile([128, 256], F32, tag="acc")
            nc.vector.tensor_copy(out=acc[:], in_=zsb[:, 0:256])
            den = wp.tile([128, 1], F32, tag="den")
            nc.vector.tensor_copy(out=den[:], in_=zsb[:, 0:1])
            e_sb = wp.tile([128, S], F32, tag="e_sb")

            uxts = {}
            ag1_cc = {}
            ag2_cc = {}
            agin2_w = {}
            hst_w = {}

            def emit_ag2(q):
                cc = nc.gpsimd.collective_compute(
                    "AllGather", mybir.AluOpType.bypass,
                    replica_groups=G2,
                    ins=[agin2[q].ap()], outs=[agbuf2[q].ap()])
                for d in agin2_w.get(q, []):
                    bass._add_dep_helper(cc.ins, d.ins, sync=True,
                                         reason="AG2 waits predicated agin2 writes")
                return cc

            def emit_din(p):
                """Load unit p's step inputs (stationary xT) into SBUF."""
                uxt = up.tile([128, U, 2, 128], BF16, tag="uxt", name=f"uxt{p}")
                src_x = xs_d.ap()[p * U:(p + 1) * U].rearrange("t a p b -> p t a b")
                if p < LAG:
                    nc.sync.dma_start(out=uxt[:], in_=src_x)
                else:
                    nc.sync.dma_start(out=uxt[:], in_=src_x, cond=(sel == 0))
                    d = nc.sync.dma_start(
                        out=uxt[:],
                        in_=agbuf1[p - LAG].ap()[0:U]
                        .rearrange("t p (a b) -> p t a b", a=2),
                        cond=(sel == 1))
                    bass._add_dep_helper(d.ins, ag1_cc[p - LAG].ins, sync=True,
                                         reason="din waits AG1")
                if p == LAG:
                    uxd = wp.tile([128, 256], F32, tag="uxd")
                    nc.vector.tensor_copy(
                        out=uxd[:], in_=uxt[:, 0].rearrange("p a b -> p (a b)"))
                    nc.sync.dma_start(out=uxdbg_d.ap(), in_=uxd[:])
                uxts[p] = uxt

            def emit_partA(t):
                p, i = t // U, t % U
                gb = pp.tile([128, 1024], F32, bufs=3, tag="gb", name=f"gb{t}")
                nc.tensor.matmul(gb[:, 0:512], ones_r[:], bias_r[:, 0:512],
                                 start=True, stop=False)
                nc.tensor.matmul(gb[:, 512:1024], ones_r[:], bias_r[:, 512:1024],
                                 start=True, stop=False)
                uxt = uxts[p]
                for kc in range(2):
                    nc.tensor.matmul(gb[:, 0:512], uxt[:, i, kc], wih[:, kc, 0],
                                     start=False, stop=False)
                    nc.tensor.matmul(gb[:, 512:1024], uxt[:, i, kc], wih[:, kc, 1],
                                     start=False, stop=False)
                return gb

            def emit_partB(t, gb):
                p, i = t // U, t % U
                # recurrent matmuls; gate layout [i(0:256) f(256:512) 2g(512:768) o(768:1024)]
                for kc in range(2):
                    nc.tensor.matmul(gb[:, 0:512], hT_s[:, kc], whh[:, kc, 0],
                                     start=False, stop=(kc == 1))
                    nc.tensor.matmul(gb[:, 512:1024], hT_s[:, kc], whh[:, kc, 1],
                                     start=False, stop=(kc == 1))
                sg = gp.tile([128, 1024], F32, tag="sg", name=f"sg{t}")
                nc.scalar.activation(sg[:, 0:768], gb[:, 0:768], AF.Sigmoid)
                nc.scalar.activation(sg[:, 768:1024], gb[:, 768:1024], AF.Sigmoid)
                # c = sig(f)*c + sig(i)*tanh(g);  tanh(g) = 2*sig(2g)-1
                v = gp.tile([128, 256], F32, tag="v", name=f"v{t}")
                nc.vector.tensor_scalar(out=v[:], in0=sg[:, 512:768],
                                        scalar1=2.0, scalar2=-1.0,
                                        op0=OP.mult, op1=OP.add)
                nc.vector.tensor_mul(out=v[:], in0=v[:], in1=sg[:, 0:256])
                nc.vector.tensor_mul(out=c_s[:], in0=c_s[:], in1=sg[:, 256:512])
                nc.vector.tensor_add(out=c_s[:], in0=c_s[:], in1=v[:])
                tc_ = gp.tile([128, 256], F32, tag="tc", name=f"tc{t}")
                nc.scalar.activation(tc_[:], c_s[:], AF.Tanh)
                h_t = gp.tile([128, 256], F32, tag="h_t", name=f"h{t}")
                nc.vector.tensor_mul(out=h_t[:], in0=tc_[:], in1=sg[:, 768:1024])
                # transpose h for the next step's stationary + the handoff
                tpq = pp.tile([128, 256], F32, bufs=1, tag="tpq", name=f"tpq{t}")
                nc.tensor.transpose(tpq[:, 0:128], h_t[:, 0:128], ident[:])
                nc.tensor.transpose(tpq[:, 128:256], h_t[:, 128:256], ident[:])
                nc.vector.tensor_copy(
                    out=hT_s[:].rearrange("p a b -> p (a b)"), in_=tpq[:])
                # ship transposed h (bf16) for the L1 consumer
                if p < NU:
                    nc.gpsimd.dma_start(
                        out=agin1[p].ap()[i],
                        in_=hT_s[:].rearrange("p a b -> p (a b)").bitcast(F32))
                # local h stores (role-shifted step index) for attention
                if p < NU:
                    d = nc.sync.dma_start(out=hstore.ap()[t], in_=h_t[:],
                                          cond=(sel == 0))
                    hst_w.setdefault(t, []).append(d)
                if p >= LAG:
                    d = nc.sync.dma_start(out=hstore.ap()[t - LAG * U], in_=h_t[:],
                                          cond=(sel == 1))
                    hst_w.setdefault(t - LAG * U, []).append(d)
                # attention projection u_t = (2*Wa1_side) @ h_t   [batch, A]
                aps = pp.tile([128, A], F32, bufs=1, tag="aps", name=f"aps{t}")
                nc.tensor.matmul(aps[:], hT_s[:, 0], wa1[:, 0],
                                 start=True, stop=False)
                nc.tensor.matmul(aps[:], hT_s[:, 1], wa1[:, 1],
                                 start=False, stop=True)
                nc.vector.tensor_copy(out=au[:, i, :], in_=aps[:])

            def emit_window(w):
                """Middle-out attention window w: blocks at MID-U*(w+1), MID+U*w."""
                for bs in (MID - U * (w + 1), MID + U * w):
                    uo = bs // U
                    ur = NU - 1 - uo
                    aot = ap_.tile([128, U, A], BF16, tag="aot", name=f"ao{w}_{bs}")
                    d1 = nc.sync.dma_start(out=aot[:], in_=agbuf2[uo].ap()[0],
                                           cond=(opos == 0))
                    d2 = nc.sync.dma_start(out=aot[:], in_=agbuf2[uo].ap()[1],
                                           cond=(opos == 1))
                    art = ap_.tile([128, U, A], BF16, tag="art", name=f"ar{w}_{bs}")
                    d3 = nc.sync.dma_start(out=art[:], in_=agbuf2[ur].ap()[1],
                                           cond=(opos == 0))
                    d4 = nc.sync.dma_start(out=art[:], in_=agbuf2[ur].ap()[0],
                                           cond=(opos == 1))
                    for d_, q_ in ((d1, uo), (d2, uo), (d3, ur), (d4, ur)):
                        bass._add_dep_helper(d_.ins, ag2_cc[q_].ins, sync=True,
                                             reason="window waits AG2")
                    asum = ap_.tile([128, U, A], F32, tag="asum", name=f"as{w}_{bs}")
                    nc.vector.tensor_add(out=asum[:], in0=aot[:],
                                         in1=art[:, ::-1, :])
                    nc.scalar.activation(asum[:], asum[:], AF.Sigmoid)
                    nc.vector.tensor_mul(
                        out=asum[:], in0=asum[:],
                        in1=wa2bc[:].unsqueeze(1).broadcast_to([128, U, A]))
                    sco = ap_.tile([128, U], F32, tag="sco", name=f"sc{w}_{bs}")
                    nc.vector.reduce_sum(out=sco[:], in_=asum[:], axis=AX.X)
                    if w == 0 and bs == MID - U:
                        zdt = wp.tile([128, U], F32, tag="zdt")
                        nc.vector.tensor_copy(out=zdt[:], in_=sco[:])
                        nc.sync.dma_start(out=zdbg_d.ap(), in_=zdt[:])
                        asdt = wp.tile([128, U, A], F32, tag="asdt")
                        nc.vector.tensor_copy(out=asdt[:], in_=asum[:])
                        nc.sync.dma_start(out=asdbg_d.ap(), in_=asdt[:])
                    nc.scalar.activation(sco[:], sco[:], AF.Sigmoid,
                                         bias=mneg[:, 0:1])
                    dt_ = ap_.tile([128, U], F32, tag="dt", name=f"dt{w}_{bs}")
                    nc.vector.tensor_scalar(out=dt_[:], in0=sco[:],
                                            scalar1=-1.0, scalar2=1.0,
                                            op0=OP.mult, op1=OP.add)
                    nc.vector.reciprocal(out=dt_[:], in_=dt_[:])
                    e_t = ap_.tile([128, U], F32, tag="e_t", name=f"e{w}_{bs}")
                    nc.vector.tensor_mul(out=e_t[:], in0=sco[:], in1=dt_[:])
                    nc.vector.tensor_copy(out=e_sb[:, bs:bs + U], in_=e_t[:])
                    dsum = ap_.tile([128, 1], F32, tag="dsum", name=f"ds{w}_{bs}")
                    nc.vector.reduce_sum(out=dsum[:], in_=e_t[:], axis=AX.X)
                    nc.vector.tensor_add(out=den[:], in0=den[:], in1=dsum[:])
                    for j in range(U):
                        hw = hp.tile([128, 256], F32, tag="hw",
                                     name=f"hw{w}_{bs}_{j}")
                        dh = nc.sync.dma_start(out=hw[:], in_=hstore.ap()[bs + j])
                        for d_ in hst_w.get(bs + j, []):
                            bass._add_dep_helper(dh.ins, d_.ins, sync=True,
                                                 reason="hw read waits hstore writes")
                        nc.vector.scalar_tensor_tensor(
                            out=acc[:], in0=hw[:], scalar=e_t[:, j:j + 1],
                            in1=acc[:], op0=OP.mult, op1=OP.add)

            # ---------------- main flat pipeline ----------------
            PF = 2
            emit_din(0)
            pend = {}
            for p in range(NP):
                if p >= 1 and p - 1 < NU:
                    ag1_cc[p - 1] = nc.gpsimd.collective_compute(
                        "AllGather", mybir.AluOpType.bypass,
                        replica_groups=G1,
                        ins=[agin1[p - 1].ap()], outs=[agbuf1[p - 1].ap()])
                if p >= LAG + 1 and p - LAG - 1 < NU:
                    ag2_cc[p - LAG - 1] = emit_ag2(p - LAG - 1)
                if p + 1 < NP:
                    emit_din(p + 1)
                if p == LAG:
                    # L1-role cores restart their recurrent state for unit 0
                    nc.sync.dma_start(out=c_s[:], in_=zdram.ap()[:, 0:256],
                                      cond=(sel == 1))
                    nc.sync.dma_start(
                        out=hT_s[:].rearrange("p a b -> p (a b)").bitcast(F32),
                        in_=zdram.ap()[:, 256:512],
                        cond=(sel == 1))
                au = ap_.tile([128, U, A], BF16, tag="au", name=f"au{p}")
                for i in range(U):
                    t = p * U + i
                    if t == 0:
                        pend[0] = emit_partA(0)
                        pend[1] = emit_partA(1)
                    emit_partB(t, pend.pop(t))
                    ta = t + PF
                    if ta < NP * U:
                        pend[ta] = emit_partA(ta)
                # ship this unit's attention projections (role-shifted index)
                if p < NU:
                    d = nc.sync.dma_start(out=agin2[p].ap(), in_=au[:],
                                          cond=(sel == 0))
                    agin2_w.setdefault(p, []).append(d)
                if p >= LAG:
                    d = nc.sync.dma_start(out=agin2[p - LAG].ap(), in_=au[:],
                                          cond=(sel == 1))
                    agin2_w.setdefault(p - LAG, []).append(d)
                w = p - WPOS0
                if 0 <= w < NW:
                    emit_window(w)

            # tail: remaining AG2s + windows + finalize
            for q in range(NP - LAG - 1, NU):
                ag2_cc[q] = emit_ag2(q)
            for w in range(NP - WPOS0, NW):
                if w >= 0:
                    emit_window(w)
            rden = ap_.tile([128, 1], F32, tag="rden")
            nc.vector.reciprocal(out=rden[:], in_=den[:])
            yt = ap_.tile([128, 256], F32, tag="yt")
            nc.vector.tensor_scalar_mul(yt[:], acc[:], rden[:, 0:1])
            nc.sync.dma_start(out=yout_d.ap(), in_=yt[:])
            hlt = ap_.tile([128, 256], F32, tag="hlt")
            nc.sync.dma_start(out=hlt[:], in_=hstore.ap()[S - 1])
            nc.sync.dma_start(out=hlast_d.ap(), in_=hlt[:])
            nc.sync.dma_start(out=den_d.ap(), in_=den[:])
            nc.sync.dma_start(out=edbg_d.ap()[:, 0:S], in_=e_sb[:])
            nc.sync.dma_start(out=acc_d.ap(), in_=acc[:])
            agt = ap_.tile([128, 256], F32, tag="agt")
            nc.gpsimd.dma_start(out=agt[:], in_=agbuf1[0].ap()[0])
            nc.sync.dma_start(out=agdbg_d.ap(), in_=agt[:])
            a2t = ap_.tile([128, 2, 16, A], F32, tag="a2t")
            nc.gpsimd.dma_start(out=a2t[:, 0], in_=agbuf2[NU - 1].ap()[0][:, 0:16])
            nc.gpsimd.dma_start(out=a2t[:, 1], in_=agbuf2[NU - 1].ap()[1][:, 0:16])
            nc.sync.dma_start(out=a2dbg_d.ap(), in_=a2t[:])

    nc.compile()
    return nc


def _prep_lstm_w(Wih, Whh, bih, bhh):
    # gate order [i, f, 2g, o]
    def reorder(M):
        return np.concatenate([M[0:256], M[256:512], 2.0 * M[512:768],
                               M[768:1024]], axis=0)

    wih_t = np.ascontiguousarray(reorder(np.asarray(Wih, np.float32)).T)
    whh_t = np.ascontiguousarray(reorder(np.asarray(Whh, np.float32)).T)
    bias = reorder((np.asarray(bih, np.float32)
                    + np.asarray(bhh, np.float32)).reshape(1024, 1)).reshape(1, 1024)

    def chunk(WT):  # [256, 1024] -> [128, kc2, half2, 512]
        return np.ascontiguousarray(
            WT.reshape(2, 128, 2, 512).transpose(1, 0, 2, 3)).astype(np.float32)

    return chunk(wih_t), chunk(whh_t), bias.astype(np.float32)


def kernel(**inputs):
    import ml_dtypes
    from concourse.bass_utils import run_bass_kernel_spmd

    BF = ml_dtypes.bfloat16
    x = np.asarray(inputs["x"], np.float32)
    Bv, S, Dv = x.shape
    NPU = (S // U + LAG) * U
    if (S, "nc") not in _BUILD_CACHE:
        _BUILD_CACHE[(S, "nc")] = _build(S)
    nc = _BUILD_CACHE[(S, "nc")]

    def prep_xs(xa):  # [B, S, D] -> [NP*U, 2, 128, B] bf16
        xs = np.zeros((NPU, 2, 128, Bv), BF)
        xs[:S] = np.ascontiguousarray(xa.transpose(1, 2, 0)).reshape(
            S, 2, 128, Bv).astype(BF)
        return xs

    xs_f = prep_xs(x)
    xs_b = prep_xs(x[:, ::-1])
    z_xs = np.zeros((NPU, 2, 128, Bv), BF)
    zw_bf = np.zeros((128, 2, 2, 512), BF)
    zw = np.zeros((128, 2, 2, 512), np.float32)
    zb = np.zeros((1, 1024), np.float32)
    zwa1 = np.zeros((128, 2, A), np.float32)

    wf0 = _prep_lstm_w(inputs["Wih_f0"], inputs["Whh_f0"], inputs["bih_f0"], inputs["bhh_f0"])
    wf1 = _prep_lstm_w(inputs["Wih_f1"], inputs["Whh_f1"], inputs["bih_f1"], inputs["bhh_f1"])
    wb0 = _prep_lstm_w(inputs["Wih_b0"], inputs["Whh_b0"], inputs["bih_b0"], inputs["bhh_b0"])
    wb1 = _prep_lstm_w(inputs["Wih_b1"], inputs["Whh_b1"], inputs["bih_b1"], inputs["bhh_b1"])

    wa1p = 2.0 * np.asarray(inputs["Wa1"], np.float32)        # [64, 512]
    # side chunks: [128, kc2, A] with wa1s[p, kc, a] = wa1p[a, side*256 + kc*128 + p]
    wa1f = np.ascontiguousarray(
        wa1p[:, 0:256].T.reshape(2, 128, A).transpose(1, 0, 2)).astype(np.float32)
    wa1b = np.ascontiguousarray(
        wa1p[:, 256:512].T.reshape(2, 128, A).transpose(1, 0, 2)).astype(np.float32)
    wa2p = (2.0 * np.asarray(inputs["Wa2"], np.float32)).reshape(1, A)
    mconst = np.float32(np.maximum(wa2p, 0.0).sum())
    mneg = np.full((1, 1), -mconst, np.float32)

    def imap(xs, w3, wih_bf, wa1s):
        wih, whh, bias = w3
        return {"xs": xs, "wih": wih_bf, "whh": whh, "bias": bias,
                "wa1s": wa1s, "wa2r": wa2p, "mneg": mneg}

    def to_bf(w):
        return np.ascontiguousarray(w.astype(BF))

    zero3 = (zw, zw, zb)
    in_maps = [
        imap(xs_f, wf0, to_bf(wf0[0]), zwa1),
        imap(z_xs, wf1, to_bf(wf1[0]), wa1f),
        imap(xs_b, wb0, to_bf(wb0[0]), zwa1),
        imap(z_xs, wb1, to_bf(wb1[0]), wa1b),
        imap(z_xs, zero3, zw_bf, zwa1),
        imap(z_xs, zero3, zw_bf, zwa1),
        imap(z_xs, zero3, zw_bf, zwa1),
        imap(z_xs, zero3, zw_bf, zwa1),
    ]
    global _last_in_maps, _last_res
    _last_in_maps = in_maps
    res = run_bass_kernel_spmd(nc, in_maps, core_ids=list(range(8)))
    _last_res = res
    out = np.concatenate([res.results[1]["yout"], res.results[3]["yout"]], axis=1)
    return out.astype(np.float32)


# revision 25
# speedup vs baseline: 1.5164x; 1.5164x over previous
"""BiLSTM (2-layer, bidirectional) encoder + attention pooling on 8 Trainium2 cores.

Flat self-timed SPMD pipeline (no tc.If blocks at all):
  core 0: layer-0 forward LSTM  (reads x, full batch 128)
  core 1: layer-1 forward LSTM + f-side attention (consumes core0 via pair-AG)
  core 2: layer-0 backward LSTM (host-reversed x)
  core 3: layer-1 backward LSTM + b-side attention
  cores 4-7: junk mirrors (participate in dummy AG pairs only)

Every core runs the *identical* instruction stream; role asymmetry is purely
data-driven: per-core weights in in_maps, plus predicated DMAs
(dma_start(cond=...)) that select input source (x vs AllGathered h) and
role-shifted store indices. Layer handoff ships transposed hidden states
(bf16) through per-unit pairwise AllGathers ([[0,1],[2,3],[4,5],[6,7]]);
the attention projections (64-dim) are exchanged between cores 1 and 3
([[1,3],...]), so each L1 core accumulates its own half of the attention
output from its locally stored h states (middle-out streaming softmax with
a fixed exp-shift M = sum(max(2*Wa2, 0)); all transcendentals are in the
sigmoid table set: Sigmoid + Tanh).
"""
import numpy as np

B, S_FULL, D, H, A = 128, 512, 256, 256, 64
U = 16            # steps per pipeline unit (AG granularity)
LAG = 2           # units of layer-0 -> layer-1 pipeline lag

_BUILD_CACHE = {}
_last_in_maps = None


def _build(S):
    import concourse.bass as bass
    from concourse import bacc
    import concourse.mybir as mybir
    from concourse.tile import TileContext
    from concourse.masks import make_identity

    F32 = mybir.dt.float32
    F32R = mybir.dt.float32r
    BF16 = mybir.dt.bfloat16
    AF = mybir.ActivationFunctionType
    OP = mybir.AluOpType
    AX = mybir.AxisListType

    NU = S // U          # real units
    NP = NU + LAG        # step positions (incl. warmup/pad junk units)
    NW = NU // 2         # attention windows
    MID = S // 2
    WPOS0 = NU // 2 + LAG + 2   # position at which window 0 is processed

    G1 = [[0, 1], [2, 3], [4, 5], [6, 7]]   # L0 -> L1 h handoff pairs
    G2 = [[0, 2], [1, 3], [4, 6], [5, 7]]   # attention u/v exchange pairs

    nc = bacc.Bacc("TRN2", target_bir_lowering=False, debug=False, num_devices=8)

    xs_d = nc.dram_tensor("xs", [NP * U, 2, 128, B], BF16, kind="ExternalInput")
    wih_d = nc.dram_tensor("wih", [128, 2, 2, 512], BF16, kind="ExternalInput")
    whh_d = nc.dram_tensor("whh", [128, 2, 2, 512], F32, kind="ExternalInput")
    bias_d = nc.dram_tensor("bias", [1, 1024], F32, kind="ExternalInput")
    wa1_d = nc.dram_tensor("wa1s", [128, 2, A], F32, kind="ExternalInput")
    wa2_d = nc.dram_tensor("wa2r", [1, A], F32, kind="ExternalInput")
    mneg_d = nc.dram_tensor("mneg", [1, 1], F32, kind="ExternalInput")
    yout_d = nc.dram_tensor("yout", [B, H], F32, kind="ExternalOutput")

    agin1 = [nc.dram_tensor(f"agin1_{q}", [U, 128, 256], BF16) for q in range(NU)]
    agbuf1 = [nc.dram_tensor(f"agbuf1_{q}", [2 * U, 128, 256], BF16) for q in range(NU)]
    agin2 = [nc.dram_tensor(f"agin2_{q}", [128, U, A], BF16) for q in range(NU)]
    agbuf2 = [nc.dram_tensor(f"agbuf2_{q}", [2, 128, U, A], BF16) for q in range(NU)]
    hstore = nc.dram_tensor("hstore", [S, 128, 256], F32)
    zdram = nc.dram_tensor("zdram", [128, 512], F32)

    with TileContext(nc) as tc:
        with tc.tile_pool(name="wpool", bufs=1) as wp, \
             tc.tile_pool(name="upool", bufs=2) as up, \
             tc.tile_pool(name="gpool", bufs=3) as gp, \
             tc.tile_pool(name="apool", bufs=2) as ap_, \
             tc.tile_pool(name="hpool", bufs=4) as hp, \
             tc.tile_pool(name="psum", bufs=1, space="PSUM") as pp:

            pid = nc.partition_id()
            sel = pid % 2            # 0: L0-style (input = xs); 1: L1-style
            opos = (pid // 2) % 2    # slot within the G2 pair

            # ---------------- prologue ----------------
            wih = wp.tile([128, 2, 2, 512], BF16, tag="wih")
            nc.sync.dma_start(out=wih[:], in_=wih_d.ap())
            whh = wp.tile([128, 2, 2, 512], F32R, tag="whh")
            nc.gpsimd.dma_start(out=whh[:], in_=whh_d.ap())
            bias_r = wp.tile([1, 1024], BF16, tag="bias_r")
            nc.gpsimd.dma_start(out=bias_r[:], in_=bias_d.ap())
            wa1 = wp.tile([128, 2, A], F32R, tag="wa1")
            nc.gpsimd.dma_start(out=wa1[:], in_=wa1_d.ap())
            wa2bc = wp.tile([128, A], F32, tag="wa2bc")
            nc.sync.dma_start(out=wa2bc[:], in_=wa2_d.ap().partition_broadcast(128))
            mneg = wp.tile([128, 1], F32, tag="mneg")
            nc.sync.dma_start(out=mneg[:], in_=mneg_d.ap().partition_broadcast(128))
            ones_f = wp.tile([1, 128], F32, tag="ones_f")
            nc.gpsimd.memset(ones_f[:], 1.0)
            ones_r = wp.tile([1, 128], BF16, tag="ones_r")
            nc.vector.tensor_copy(out=ones_r[:], in_=ones_f[:])
            ident = wp.tile([128, 128], F32, tag="ident")
            make_identity(nc, ident[:])

            zsb = wp.tile([128, 512], F32, tag="zsb")
            nc.gpsimd.memset(zsb[:], 0.0)
            nc.sync.dma_start(out=zdram.ap(), in_=zsb[:])
            hT_s = wp.tile([128, 2, 128], F32R, tag="hT_s")
            nc.vector.tensor_copy(out=hT_s[:].rearrange("p a b -> p (a b)"),
                                  in_=zsb[:, 0:256])
            c_s = wp.tile([128, 256], F32, tag="c_s")
            nc.vector.tensor_copy(out=c_s[:], in_=zsb[:, 0:256])
            acc = wp.tile([128, 256], F32, tag="acc")
            nc.vector.tensor_copy(out=acc[:], in_=zsb[:, 0:256])
            den = wp.tile([128, 1], F32, tag="den")
            nc.vector.tensor_copy(out=den[:], in_=zsb[:, 0:1])

            uxts = {}
            ag1_cc = {}
            ag2_cc = {}
            agin2_w = {}
            hst_w = {}

            def emit_ag2(q):
                cc = nc.gpsimd.collective_compute(
                    "AllGather", mybir.AluOpType.bypass,
                    replica_groups=G2,
                    ins=[agin2[q].ap()], outs=[agbuf2[q].ap()])
                for d in agin2_w.get(q, []):
                    bass._add_dep_helper(cc.ins, d.ins, sync=True,
                                         reason="AG2 waits predicated agin2 writes")
                return cc

            def emit_din(p):
                """Load unit p's step inputs (stationary xT) into SBUF."""
                uxt = up.tile([128, U, 2, 128], BF16, tag="uxt", name=f"uxt{p}")
                src_x = xs_d.ap()[p * U:(p + 1) * U].rearrange("t a p b -> p t a b")
                if p < LAG:
                    nc.sync.dma_start(out=uxt[:], in_=src_x)
                else:
                    nc.sync.dma_start(out=uxt[:], in_=src_x, cond=(sel == 0))
                    d = nc.sync.dma_start(
                        out=uxt[:],
                        in_=agbuf1[p - LAG].ap()[0:U]
                        .rearrange("t p (a b) -> p t a b", a=2),
                        cond=(sel == 1))
                    bass._add_dep_helper(d.ins, ag1_cc[p - LAG].ins, sync=True,
                                         reason="din waits AG1")
                uxts[p] = uxt

            def emit_partA(t):
                p, i = t // U, t % U
                gb = pp.tile([128, 1024], F32, bufs=3, tag="gb", name=f"gb{t}")
                nc.tensor.matmul(gb[:, 0:512], ones_r[:], bias_r[:, 0:512],
                                 start=True, stop=False)
                nc.tensor.matmul(gb[:, 512:1024], ones_r[:], bias_r[:, 512:1024],
                                 start=True, stop=False)
                uxt = uxts[p]
                for kc in range(2):
                    nc.tensor.matmul(gb[:, 0:512], uxt[:, i, kc], wih[:, kc, 0],
                                     start=False, stop=False)
                    nc.tensor.matmul(gb[:, 512:1024], uxt[:, i, kc], wih[:, kc, 1],
                                     start=False, stop=False)
                return gb

            def emit_partB(t, gb):
                p, i = t // U, t % U
                # recurrent matmuls; gate layout [i(0:256) f(256:512) 2g(512:768) o(768:1024)]
                for kc in range(2):
                    nc.tensor.matmul(gb[:, 0:512], hT_s[:, kc], whh[:, kc, 0],
                                     start=False, stop=(kc == 1))
                    nc.tensor.matmul(gb[:, 512:1024], hT_s[:, kc], whh[:, kc, 1],
                                     start=False, stop=(kc == 1))
                sg = gp.tile([128, 1024], F32, tag="sg", name=f"sg{t}")
                nc.scalar.activation(sg[:, 0:768], gb[:, 0:768], AF.Sigmoid)
                nc.scalar.activation(sg[:, 768:1024], gb[:, 768:1024], AF.Sigmoid)
                # c = sig(f)*c + sig(i)*tanh(g);  tanh(g) = 2*sig(2g)-1
                v = gp.tile([128, 256], F32, tag="v", name=f"v{t}")
                nc.vector.tensor_scalar(out=v[:], in0=sg[:, 512:768],
                                        scalar1=2.0, scalar2=-1.0,
                                        op0=OP.mult, op1=OP.add)
                nc.vector.tensor_mul(out=v[:], in0=v[:], in1=sg[:, 0:256])
                nc.vector.tensor_mul(out=c_s[:], in0=c_s[:], in1=sg[:, 256:512])
                nc.vector.tensor_add(out=c_s[:], in0=c_s[:], in1=v[:])
                tc_ = gp.tile([128, 256], F32, tag="tc", name=f"tc{t}")
                nc.scalar.activation(tc_[:], c_s[:], AF.Tanh)
                h_t = gp.tile([128, 256], F32, tag="h_t", name=f"h{t}")
                nc.vector.tensor_mul(out=h_t[:], in0=tc_[:], in1=sg[:, 768:1024])
                # transpose h for the next step's stationary + the handoff
                tpq = pp.tile([128, 256], F32, bufs=1, tag="tpq", name=f"tpq{t}")
                nc.tensor.transpose(tpq[:, 0:128], h_t[:, 0:128], ident[:])
                nc.tensor.transpose(tpq[:, 128:256], h_t[:, 128:256], ident[:])
                nc.vector.tensor_copy(
                    out=hT_s[:].rearrange("p a b -> p (a b)"), in_=tpq[:])
                # ship transposed h (bf16) for the L1 consumer
                if p < NU:
                    nc.gpsimd.dma_start(
                        out=agin1[p].ap()[i],
                        in_=hT_s[:].rearrange("p a b -> p (a b)").bitcast(F32))
                # local h stores (role-shifted step index) for attention
                if p < NU:
                    d = nc.sync.dma_start(out=hstore.ap()[t], in_=h_t[:],
                                          cond=(sel == 0))
                    hst_w.setdefault(t, []).append(d)
                if p >= LAG:
                    d = nc.sync.dma_start(out=hstore.ap()[t - LAG * U], in_=h_t[:],
                                          cond=(sel == 1))
                    hst_w.setdefault(t - LAG * U, []).append(d)
                # attention projection u_t = (2*Wa1_side) @ h_t   [batch, A]
                aps = pp.tile([128, A], F32, bufs=1, tag="aps", name=f"aps{t}")
                nc.tensor.matmul(aps[:], hT_s[:, 0], wa1[:, 0],
                                 start=True, stop=False)
                nc.tensor.matmul(aps[:], hT_s[:, 1], wa1[:, 1],
                                 start=False, stop=True)
                nc.vector.tensor_copy(out=au[:, i, :], in_=aps[:])

            def emit_window(w):
                """Middle-out attention window w: blocks at MID-U*(w+1), MID+U*w."""
                for bs in (MID - U * (w + 1), MID + U * w):
                    uo = bs // U
                    ur = NU - 1 - uo
                    aot = ap_.tile([128, U, A], BF16, tag="aot", name=f"ao{w}_{bs}")
                    d1 = nc.sync.dma_start(out=aot[:], in_=agbuf2[uo].ap()[0],
                                           cond=(opos == 0))
                    d2 = nc.sync.dma_start(out=aot[:], in_=agbuf2[uo].ap()[1],
                                           cond=(opos == 1))
                    art = ap_.tile([128, U, A], BF16, tag="art", name=f"ar{w}_{bs}")
                    d3 = nc.sync.dma_start(out=art[:], in_=agbuf2[ur].ap()[1],
                                           cond=(opos == 0))
                    d4 = nc.sync.dma_start(out=art[:], in_=agbuf2[ur].ap()[0],
                                           cond=(opos == 1))
                    for d_, q_ in ((d1, uo), (d2, uo), (d3, ur), (d4, ur)):
                        bass._add_dep_helper(d_.ins, ag2_cc[q_].ins, sync=True,
                                             reason="window waits AG2")
                    asum = ap_.tile([128, U, A], F32, tag="asum", name=f"as{w}_{bs}")
                    nc.vector.tensor_add(out=asum[:], in0=aot[:],
                                         in1=art[:, ::-1, :])
                    nc.scalar.activation(asum[:], asum[:], AF.Sigmoid)
                    nc.vector.tensor_mul(
                        out=asum[:], in0=asum[:],
                        in1=wa2bc[:].unsqueeze(1).broadcast_to([128, U, A]))
                    sco = ap_.tile([128, U], F32, tag="sco", name=f"sc{w}_{bs}")
                    nc.vector.reduce_sum(out=sco[:], in_=asum[:], axis=AX.X)
                    nc.scalar.activation(sco[:], sco[:], AF.Sigmoid,
                                         bias=mneg[:, 0:1])
                    dt_ = ap_.tile([128, U], F32, tag="dt", name=f"dt{w}_{bs}")
                    nc.vector.tensor_scalar(out=dt_[:], in0=sco[:],
                                            scalar1=-1.0, scalar2=1.0,
                                            op0=OP.mult, op1=OP.add)
                    nc.vector.reciprocal(out=dt_[:], in_=dt_[:])
                    e_t = ap_.tile([128, U], F32, tag="e_t", name=f"e{w}_{bs}")
                    nc.vector.tensor_mul(out=e_t[:], in0=sco[:], in1=dt_[:])
                    dsum = ap_.tile([128, 1], F32, tag="dsum", name=f"ds{w}_{bs}")
                    nc.vector.reduce_sum(out=dsum[:], in_=e_t[:], axis=AX.X)
                    nc.vector.tensor_add(out=den[:], in0=den[:], in1=dsum[:])
                    for j in range(U):
                        hw = hp.tile([128, 256], F32, tag="hw",
                                     name=f"hw{w}_{bs}_{j}")
                        dh = nc.sync.dma_start(out=hw[:], in_=hstore.ap()[bs + j])
                        for d_ in hst_w.get(bs + j, []):
                            bass._add_dep_helper(dh.ins, d_.ins, sync=True,
                                                 reason="hw read waits hstore writes")
                        nc.vector.scalar_tensor_tensor(
                            out=acc[:], in0=hw[:], scalar=e_t[:, j:j + 1],
                            in1=acc[:], op0=OP.mult, op1=OP.add)

            # ---------------- main flat pipeline ----------------
            PF = 2
            emit_din(0)
            pend = {}
            for p in range(NP):
                if p >= 1 and p - 1 < NU:
                    ag1_cc[p - 1] = nc.gpsimd.collective_compute(
                        "AllGather", mybir.AluOpType.bypass,
                        replica_groups=G1,
                        ins=[agin1[p - 1].ap()], outs=[agbuf1[p - 1].ap()])
                if p >= LAG + 1 and p - LAG - 1 < NU:
                    ag2_cc[p - LAG - 1] = emit_ag2(p - LAG - 1)
                if p + 1 < NP:
                    emit_din(p + 1)
                if p == LAG:
                    # L1-role cores restart their recurrent state for unit 0
                    nc.sync.dma_start(out=c_s[:], in_=zdram.ap()[:, 0:256],
                                      cond=(sel == 1))
                    nc.sync.dma_start(
                        out=hT_s[:].rearrange("p a b -> p (a b)").bitcast(F32),
                        in_=zdram.ap()[:, 256:512],
                        cond=(sel == 1))
                au = ap_.tile([128, U, A], BF16, tag="au", name=f"au{p}")
                for i in range(U):
                    t = p * U + i
                    if t == 0:
                        pend[0] = emit_partA(0)
                        pend[1] = emit_partA(1)
                    emit_partB(t, pend.pop(t))
                    ta = t + PF
                    if ta < NP * U:
                        pend[ta] = emit_partA(ta)
                # ship this unit's attention projections (role-shifted index)
                if p < NU:
                    d = nc.sync.dma_start(out=agin2[p].ap(), in_=au[:],
                                          cond=(sel == 0))
                    agin2_w.setdefault(p, []).append(d)
                if p >= LAG:
                    d = nc.sync.dma_start(out=agin2[p - LAG].ap(), in_=au[:],
                                          cond=(sel == 1))
                    agin2_w.setdefault(p - LAG, []).append(d)
                w = p - WPOS0
                if 0 <= w < NW:
                    emit_window(w)

            # tail: remaining AG2s + windows + finalize
            for q in range(NP - LAG - 1, NU):
                ag2_cc[q] = emit_ag2(q)
            for w in range(NP - WPOS0, NW):
                if w >= 0:
                    emit_window(w)
            rden = ap_.tile([128, 1], F32, tag="rden")
            nc.vector.reciprocal(out=rden[:], in_=den[:])
            yt = ap_.tile([128, 256], F32, tag="yt")
            nc.vector.tensor_scalar_mul(yt[:], acc[:], rden[:, 0:1])
            nc.sync.dma_start(out=yout_d.ap(), in_=yt[:])

    nc.compile()
    return nc


def _prep_lstm_w(Wih, Whh, bih, bhh):
    # gate order [i, f, 2g, o]
    def reorder(M):
        return np.concatenate([M[0:256], M[256:512], 2.0 * M[512:768],
                               M[768:1024]], axis=0)

    wih_t = np.ascontiguousarray(reorder(np.asarray(Wih, np.float32)).T)
    whh_t = np.ascontiguousarray(reorder(np.asarray(Whh, np.float32)).T)
    bias = reorder((np.asarray(bih, np.float32)
                    + np.asarray(bhh, np.float32)).reshape(1024, 1)).reshape(1, 1024)

    def chunk(WT):  # [256, 1024] -> [128, kc2, half2, 512]
        return np.ascontiguousarray(
            WT.reshape(2, 128, 2, 512).transpose(1, 0, 2, 3)).astype(np.float32)

    return chunk(wih_t), chunk(whh_t), bias.astype(np.float32)


def kernel(**inputs):
    import ml_dtypes
    from concourse.bass_utils import run_bass_kernel_spmd

    BF = ml_dtypes.bfloat16
    x = np.asarray(inputs["x"], np.float32)
    Bv, S, Dv = x.shape
    NPU = (S // U + LAG) * U
    if (S, "nc") not in _BUILD_CACHE:
        _BUILD_CACHE[(S, "nc")] = _build(S)
    nc = _BUILD_CACHE[(S, "nc")]

    def prep_xs(xa):  # [B, S, D] -> [NP*U, 2, 128, B] bf16
        xs = np.zeros((NPU, 2, 128, Bv), BF)
        xs[:S] = np.ascontiguousarray(xa.transpose(1, 2, 0)).reshape(
            S, 2, 128, Bv).astype(BF)
        return xs

    xs_f = prep_xs(x)
    xs_b = prep_xs(x[:, ::-1])
    z_xs = np.zeros((NPU, 2, 128, Bv), BF)
    zw_bf = np.zeros((128, 2, 2, 512), BF)
    zw = np.zeros((128, 2, 2, 512), np.float32)
    zb = np.zeros((1, 1024), np.float32)
    zwa1 = np.zeros((128, 2, A), np.float32)

    wf0 = _prep_lstm_w(inputs["Wih_f0"], inputs["Whh_f0"], inputs["bih_f0"], inputs["bhh_f0"])
    wf1 = _prep_lstm_w(inputs["Wih_f1"], inputs["Whh_f1"], inputs["bih_f1"], inputs["bhh_f1"])
    wb0 = _prep_lstm_w(inputs["Wih_b0"], inputs["Whh_b0"], inputs["bih_b0"], inputs["bhh_b0"])
    wb1 = _prep_lstm_w(inputs["Wih_b1"], inputs["Whh_b1"], inputs["bih_b1"], inputs["bhh_b1"])

    wa1p = 2.0 * np.asarray(inputs["Wa1"], np.float32)        # [64, 512]
    # side chunks: [128, kc2, A] with wa1s[p, kc, a] = wa1p[a, side*256 + kc*128 + p]
    wa1f = np.ascontiguousarray(
        wa1p[:, 0:256].T.reshape(2, 128, A).transpose(1, 0, 2)).astype(np.float32)
    wa1b = np.ascontiguousarray(
        wa1p[:, 256:512].T.reshape(2, 128, A).transpose(1, 0, 2)).astype(np.float32)
    wa2p = (2.0 * np.asarray(inputs["Wa2"], np.float32)).reshape(1, A)
    mconst = np.float32(np.maximum(wa2p, 0.0).sum())
    mneg = np.full((1, 1), -mconst, np.float32)

    def imap(xs, w3, wih_bf, wa1s):
        wih, whh, bias = w3
        return {"xs": xs, "wih": wih_bf, "whh": whh, "bias": bias,
                "wa1s": wa1s, "wa2r": wa2p, "mneg": mneg}

    def to_bf(w):
        return np.ascontiguousarray(w.astype(BF))

    zero3 = (zw, zw, zb)
    in_maps = [
        imap(xs_f, wf0, to_bf(wf0[0]), zwa1),
        imap(z_xs, wf1, to_bf(wf1[0]), wa1f),
        imap(xs_b, wb0, to_bf(wb0[0]), zwa1),
        imap(z_xs, wb1, to_bf(wb1[0]), wa1b),
        imap(z_xs, zero3, zw_bf, zwa1),
        imap(z_xs, zero3, zw_bf, zwa1),
        imap(z_xs, zero3, zw_bf, zwa1),
        imap(z_xs, zero3, zw_bf, zwa1),
    ]
    global _last_in_maps, _last_res
    _last_in_maps = in_maps
    res = run_bass_kernel_spmd(nc, in_maps, core_ids=list(range(8)))
    _last_res = res
    out = np.concatenate([res.results[1]["yout"], res.results[3]["yout"]], axis=1)
    return out.astype(np.float32)
